# revision 2
# baseline (speedup 1.0000x reference)
"""Trainium2 Bass kernel for an attention layer whose math collapses.

The module computes softmax over a size-1 axis, so the attention weights
are exactly 1.0 and the output is context[b, 0, d] = sum_t a[b, t, d].
The MLP branch (W1, b1, W2, b2) and s_prev never affect the output.

Strategy: pure data parallel over the batch axis; each of the 8 cores
reduces its [16, 512, 512] shard over the time axis. Memory-bound:
~16 MiB HBM read per core. Trace-measured steady-state DMA rate with
full-width transfers is ~420 GB/s (both HWDGE rings interleaving onto
all 16 SDMA engines / SBUF AXI ports), so the byte-stream floor is
~40 us plus ~7 us fixed bass preamble plus a short tail.

Kernel shape (per core):
  - 16 slabs of 1 MiB (one batch each), DMA'd as [128 partitions x
    8 KiB contiguous]. Even slabs on the SP HWDGE ring, odd slabs on
    the Activation ring, so per-DMA fixed costs overlap. Every slab
    spans all 128 partitions: 32-partition "early start" pieces only
    reach 8 of the 16 SBUF AXI ports and run the first ~12 us of the
    stream at half rate (trace-verified), so they are never used.
  - Per slab: one in-place fp32 halving add (2048 -> 1024), then a
    second add (1024 -> 512) that writes bf16 to a per-slab scratch.
    GPSIMD folds the even (SP-ring) slabs, DVE the odd slabs; both
    keep up with the ~5 us per-ring slab cadence. Same-engine chained
    adds need a semaphore handshake (deep pipelines, no RAW interlock).
  - One bf16 matmul per slab against the preamble's constant bf16 ones
    [128, 1] reduces across partitions into a psum row (fp32 accum).
    bf16 avoids the fp32 LOW/HIGH matmul split (2x ~1 us -> ~0.4 us)
    and is well within the 2e-2 tolerance (~1e-3 observed). Eight psum
    banks hold 2 slab results each at partition offsets {0, 32}.
  - The last slab of each ring (14 on SP, 15 on ACT) is loaded as two
    half-MiB DMAs; each half needs a single fold add and its own
    accumulating matmul, which shrinks the post-last-byte serial chain.
  - ACT bounces each psum row to SBUF (DMA cannot read PSUM); per-slab
    2 KiB stores overlap all but the last store's latency.

Raw Bass (not Tile): with per-DMA completion semaphores every wait is a
standalone single-condition instruction and Tile's tail barriers are
avoided.
"""

from contextlib import ExitStack

import numpy as np

B, TX, D = 128, 512, 512
N_CORES = 8
NB = B // N_CORES   # 16 batches per core
P = 128             # SBUF partitions
NSLAB = 16          # 1 MiB DMA slabs per core (= one batch per slab)
FPP = NB * TX * D // (NSLAB * P)  # f32 per partition per slab = 2048

_CACHE: dict = {}


def _build_bass():
    import concourse.bass as bass
    import concourse.mybir as mybir

    f32 = mybir.dt.float32
    bf16 = mybir.dt.bfloat16
    add = mybir.AluOpType.add
    nc = bass.Bass("TRN2")
    a = nc.dram_tensor("a", [NB, TX, D], f32, kind="ExternalInput")
    out = nc.dram_tensor("out", [NB, D], f32, kind="ExternalOutput")

    ones = nc.const_aps.aps[(bf16, 1.0)]  # preamble-initialized [128, 1]
    a_sl = a.rearrange("b t d -> (b t d)").rearrange(
        "(g p f) -> g p f", g=NSLAB, p=P
    )

    with ExitStack() as ctx:
        abuf = ctx.enter_context(nc.sbuf_tensor([P, NSLAB * FPP], f32))
        # bf16 fold-2 outputs: one 512-wide slot per matmul operand
        # (14 full slabs + 4 half-slabs).
        bbuf = ctx.enter_context(nc.sbuf_tensor([P, 18 * D], bf16))
        ost = ctx.enter_context(nc.sbuf_tensor([1, NB * D], f32))
        psb = [
            ctx.enter_context(nc.psum_tensor(f"ps{i}", [64, D], f32))
            for i in range(8)
        ]
        # One completion semaphore per DMA: concurrent DMA completions
        # are unordered, so a shared counting sem would be racy.
        ld_sems = [
            ctx.enter_context(nc.semaphore(f"ld_sem{g}")) for g in range(NSLAB)
        ]
        fold_sems = [
            ctx.enter_context(nc.semaphore(f"fold_sem{g}")) for g in range(NSLAB)
        ]
        red_sems = [
            ctx.enter_context(nc.semaphore(f"red_sem{g}")) for g in range(NSLAB)
        ]
        st_sems = [
            ctx.enter_context(nc.semaphore(f"st_sem{g}")) for g in range(NSLAB)
        ]
        pe_sem = ctx.enter_context(nc.semaphore("pe_sem"))
        cp_sem = ctx.enter_context(nc.semaphore("cp_sem"))
        block = ctx.enter_context(nc.Block(no_gpsimd_drain=True))

        abuf_t = abuf[:].rearrange("p (g f) -> p g f", g=NSLAB)
        bbuf_t = bbuf[:].rearrange("p (s f) -> p s f", s=18)
        hl_sems = {
            (g, h): ctx.enter_context(nc.semaphore(f"hl{g}_{h}"))
            for g in (14, 15)
            for h in (0, 1)
        }
        hr_sems = {
            (g, h): ctx.enter_context(nc.semaphore(f"hr{g}_{h}"))
            for g in (14, 15)
            for h in (0, 1)
        }
        HF = FPP // 2  # f32 per partition per half-slab = 1024
        a_hl = a.rearrange("b t d -> (b t d)").rearrange(
            "(x p f) -> x p f", x=2 * NSLAB, p=P
        )
        # bf16 scratch slot per matmul operand.
        BSLOT = {g: g for g in range(14)}
        BSLOT.update({(14, 0): 14, (15, 0): 15, (14, 1): 16, (15, 1): 17})
        # PE processes matmuls in this order (approximate arrival order).
        PE_ORDER = list(range(14)) + [(14, 0), (15, 0), (14, 1), (15, 1)]

        def fold_slab(eng, g):
            """fp32 in-place add 2048 -> 1024, then 1024 -> 512 written
            as bf16 to the slab's scratch slot. Same-engine RAW needs an
            explicit sem handshake per step."""
            eng.wait_ge(ld_sems[g], 16)
            sl = abuf_t[:, g]
            h = FPP // 2
            eng.tensor_tensor(sl[:, 0:h], sl[:, 0:h], sl[:, h : 2 * h], add).then_inc(
                fold_sems[g], 1
            )
            eng.wait_ge(fold_sems[g], 1)
            h = FPP // 4
            eng.tensor_tensor(
                bbuf_t[:, BSLOT[g]], sl[:, 0:h], sl[:, h : 2 * h], add
            ).then_inc(red_sems[g], 1)

        def fold_half(eng, g, h):
            """One add folds a half-slab 1024 -> 512, bf16 out."""
            eng.wait_ge(hl_sems[(g, h)], 16)
            sl = abuf_t[:, g]
            o = h * HF
            eng.tensor_tensor(
                bbuf_t[:, BSLOT[(g, h)]], sl[:, o : o + D], sl[:, o + D : o + 2 * D], add
            ).then_inc(hr_sems[(g, h)], 1)

        @block.sync
        def _(sync):
            for g in range(0, NSLAB - 2, 2):
                sync.dma_start(out=abuf_t[:, g], in_=a_sl[g]).then_inc(ld_sems[g], 16)
            for h in (0, 1):
                sync.dma_start(
                    out=abuf_t[:, 14, h * HF : (h + 1) * HF], in_=a_hl[28 + h]
                ).then_inc(hl_sems[(14, h)], 16)
            # Per-slab 2 KiB stores: all but the last store's latency
            # overlaps with remaining compute.
            for g in range(NSLAB):
                sync.wait_ge(cp_sem, g + 1)
                sync.dma_start(
                    out=out[g : g + 1, :], in_=ost[0:1, g * D : (g + 1) * D]
                ).then_inc(st_sems[g], 16)
            for g in range(NSLAB):
                sync.wait_ge(st_sems[g], 16)

        @block.scalar
        def _(scalar):
            # Second HWDGE ring (Activation sequencer) for the odd slabs.
            for g in range(1, NSLAB - 2, 2):
                scalar.dma_start(out=abuf_t[:, g], in_=a_sl[g]).then_inc(
                    ld_sems[g], 16
                )
            for h in (0, 1):
                scalar.dma_start(
                    out=abuf_t[:, 15, h * HF : (h + 1) * HF], in_=a_hl[30 + h]
                ).then_inc(hl_sems[(15, h)], 16)
            # ACT also bounces finished psum rows to SBUF (DMA cannot
            # read PSUM; DVE/GPSIMD are busy folding slabs).
            for g in range(NSLAB):
                off = 32 * (g % 2)
                n_mm = (g + 1) if g < 14 else len(PE_ORDER) - (1 - (g - 14))
                scalar.wait_ge(pe_sem, n_mm)
                scalar.copy(
                    ost[:, g * D : (g + 1) * D], psb[g // 2][off : off + 1, :]
                ).then_inc(cp_sem, 1)

        @block.gpsimd
        def _(gpsimd):
            for g in range(0, NSLAB - 2, 2):
                fold_slab(gpsimd, g)

        @block.vector
        def _(vector):
            for g in range(1, NSLAB - 2, 2):
                fold_slab(vector, g)
            for g, h in ((14, 0), (15, 0), (14, 1), (15, 1)):
                fold_half(vector, g, h)

        @block.tensor
        def _(tensor):
            for item in PE_ORDER:
                if isinstance(item, int):
                    g, first, last = item, True, True
                    tensor.wait_ge(red_sems[g], 1)
                else:
                    g, h = item
                    first, last = (h == 0), (h == 1)
                    tensor.wait_ge(hr_sems[(g, h)], 1)
                off = 32 * (g % 2)
                tensor.matmul(
                    psb[g // 2][off : off + 1, :],
                    lhsT=ones[:, 0:1],
                    rhs=bbuf_t[:, BSLOT[item]],
                    start=first,
                    stop=last,
                ).then_inc(pe_sem, 1)

    return nc


def _get_bass():
    if "nc" not in _CACHE:
        _CACHE["nc"] = _build_bass()
    return _CACHE["nc"]


def run_spmd(a, **spmd_kwargs):
    """Run the SPMD kernel on all 8 cores; returns (full_output, BassKernelResults)."""
    from concourse.bass_utils import run_bass_kernel_spmd

    nc = _get_bass()
    a = np.ascontiguousarray(np.asarray(a), dtype=np.float32)
    assert a.shape == (B, TX, D), a.shape
    in_maps = [{"a": a[k * NB : (k + 1) * NB]} for k in range(N_CORES)]
    res = run_bass_kernel_spmd(nc, in_maps, list(range(N_CORES)), **spmd_kwargs)
    out = np.concatenate([res.results[k]["out"] for k in range(N_CORES)], axis=0)
    return out.reshape(B, 1, D).astype(np.float32), res


def kernel(a, s_prev=None, W1=None, b1=None, W2=None, b2=None, **_unused):
    out, _ = run_spmd(a)
    return out


# revision 4
# speedup vs baseline: 1.1268x; 1.1268x over previous
"""Trainium2 Bass kernel for an attention layer whose math collapses.

The module computes softmax over a size-1 axis, so the attention weights
are exactly 1.0 and the output is context[b, 0, d] = sum_t a[b, t, d].
The MLP branch (W1, b1, W2, b2) and s_prev never affect the output.

Strategy: pure data parallel over the batch axis; each of the 8 cores
reduces its [16, 512, 512] shard over the time axis. Memory-bound:
~16 MiB HBM read per core; trace-measured steady-state DMA rate with
full-width transfers is ~420 GB/s (both HWDGE rings interleaving onto
all 16 SDMA engines / SBUF AXI ports) -> ~40 us byte stream plus fixed
framework pre/postamble and a short tail.

Kernel shape (per core), tuned against perfetto traces:
  - 14 slabs of 1 MiB (one batch each) + a 3-piece endgame slab per
    ring, DMA'd as [128 partitions x contiguous bytes]. Even batches on
    the SP HWDGE ring, odd batches on the Activation ring. Every
    transfer spans all 128 partitions: 32-partition "early start"
    pieces only reach 8 of the 16 SBUF AXI ports and run the first
    ~12 us of the stream at half rate (trace-verified), so they are
    never used.
  - Per slab, DVE does ONE fp32 -> bf16 cast (~1.2 us) into a rotating
    4-slot bf16 scratch (a previous in-place bitcast variant raced the
    cast's read stream and corrupted results; the scratch slot is
    guarded by the matmul-progress sem). No fold adds: the PE reduces
    both the partition axis and the 4-rows-per-partition axis with 4
    accumulating bf16 matmuls (ones [128,1] lhsT, N=512, fp32 PSUM
    accum). bf16 keeps PE at ~0.4-0.6 us/matmul (fp32 would LOW/HIGH-
    split to ~2 us) and the rounding error (~5e-4 rel) is far inside
    the 2e-2 gate. GPSIMD does nothing: concurrent GpSimd tensor ops
    stall DVE ops 2-4x (shared-port lock, trace-verified).
  - The last batch of each ring (14 on SP, 15 on ACT) is loaded as a
    half then two quarter DMAs, so the post-last-byte serial chain is
    one 512-element cast (~0.35 us) + one matmul + psum bounce + store.
  - ACT bounces each psum row to SBUF (DMA cannot read PSUM); per-slab
    2 KiB stores overlap all but the last store's latency.
  - The ~7 us end-of-kernel postamble (walrus resets all 256 HW
    semaphores round-robin across engines, inside the measured window)
    is framework-fixed; kernel-side sem slimming cannot shrink it.

Raw Bass (not Tile): with per-DMA completion semaphores every wait is a
standalone single-condition instruction and Tile's tail barriers are
avoided.
"""

from contextlib import ExitStack

import numpy as np

B, TX, D = 128, 512, 512
N_CORES = 8
NB = B // N_CORES   # 16 batches per core
P = 128             # SBUF partitions
NSLAB = 16          # 1 MiB slab groups per core (= one batch per slab)
FPP = NB * TX * D // (NSLAB * P)  # f32 per partition per slab = 2048
NSLOT = 4           # rotating bf16 scratch slots

_CACHE: dict = {}


def _build_bass():
    import concourse.bass as bass
    import concourse.mybir as mybir

    f32 = mybir.dt.float32
    bf16 = mybir.dt.bfloat16
    nc = bass.Bass("TRN2")
    a = nc.dram_tensor("a", [NB, TX, D], f32, kind="ExternalInput")
    out = nc.dram_tensor("out", [NB, D], f32, kind="ExternalOutput")

    ones = nc.const_aps.aps[(bf16, 1.0)]  # preamble-initialized [128, 1]
    a_sl = a.rearrange("b t d -> (b t d)").rearrange(
        "(g p f) -> g p f", g=NSLAB, p=P
    )

    with ExitStack() as ctx:
        abuf = ctx.enter_context(nc.sbuf_tensor([P, NSLAB * FPP], f32))
        bbuf = ctx.enter_context(nc.sbuf_tensor([P, NSLOT * FPP], bf16))
        ost = ctx.enter_context(nc.sbuf_tensor([1, NB * D], f32))
        psb = [
            ctx.enter_context(nc.psum_tensor(f"ps{i}", [64, D], f32))
            for i in range(8)
        ]
        # One completion semaphore per load DMA: concurrent DMA
        # completions are unordered, so a shared counting sem is racy.
        ld_sems = [
            ctx.enter_context(nc.semaphore(f"ld_sem{g}"))
            for g in range(NSLAB - 2)
        ]
        # Endgame pieces: (batch, piece) with piece 0 = half (1024 f32),
        # pieces 1, 2 = quarters (512 f32 each).
        eg_sems = {
            (g, q): ctx.enter_context(nc.semaphore(f"eg{g}_{q}"))
            for g in (14, 15)
            for q in (0, 1, 2)
        }
        # DVE cast counter (program-order), matmul item counter (one inc
        # per fully-matmul'd cast item, guards scratch-slot reuse and
        # gates bounces), bounce counter, store counter (all stores inc
        # one sem; the final wait needs the exact total 16*16, which is
        # order-independent).
        vred = ctx.enter_context(nc.semaphore("vred"))
        pe_sem = ctx.enter_context(nc.semaphore("pe_sem"))
        cp_sem = ctx.enter_context(nc.semaphore("cp_sem"))
        st_sem = ctx.enter_context(nc.semaphore("st_sem"))
        block = ctx.enter_context(nc.Block(no_gpsimd_drain=True))

        abuf_t = abuf[:].rearrange("p (g f) -> p g f", g=NSLAB)
        bbuf_t = bbuf[:].rearrange("p (s f) -> p s f", s=NSLOT)
        HF = FPP // 2   # 1024
        QF = FPP // 4   # 512
        # Endgame piece offsets/lengths in f32 elements within the slab.
        EG_OFF = {0: (0, HF), 1: (HF, QF), 2: (HF + QF, QF)}
        a_q = a.rearrange("b t d -> (b t d)").rearrange(
            "(x p f) -> x p f", x=4 * NSLAB, p=P
        )  # 256 KiB quarters: slab g = quarters 4g..4g+3
        a_hl = a.rearrange("b t d -> (b t d)").rearrange(
            "(x p f) -> x p f", x=2 * NSLAB, p=P
        )  # 512 KiB halves: slab g = halves 2g, 2g+1
        # DVE cast order = approximate arrival order; PE and the slot-
        # reuse guard follow the same order via counting sems.
        CAST_ORDER = list(range(14)) + [
            (14, 0), (15, 0), (14, 1), (15, 1), (14, 2), (15, 2)
        ]

        @block.sync
        def _(sync):
            for g in range(0, NSLAB - 2, 2):
                sync.dma_start(out=abuf_t[:, g], in_=a_sl[g]).then_inc(ld_sems[g], 16)
            sync.dma_start(
                out=abuf_t[:, 14, 0:HF], in_=a_hl[28]
            ).then_inc(eg_sems[(14, 0)], 16)
            for q in (1, 2):
                o, n = EG_OFF[q]
                sync.dma_start(
                    out=abuf_t[:, 14, o : o + n], in_=a_q[56 + 1 + q]
                ).then_inc(eg_sems[(14, q)], 16)
            # Per-slab 2 KiB stores: all but the last store's latency
            # overlaps with remaining compute.
            for g in range(NSLAB):
                sync.wait_ge(cp_sem, g + 1)
                sync.dma_start(
                    out=out[g : g + 1, :], in_=ost[0:1, g * D : (g + 1) * D]
                ).then_inc(st_sem, 16)
            sync.wait_ge(st_sem, 16 * NSLAB)

        @block.scalar
        def _(scalar):
            # Second HWDGE ring (Activation sequencer) for the odd slabs.
            for g in range(1, NSLAB - 2, 2):
                scalar.dma_start(out=abuf_t[:, g], in_=a_sl[g]).then_inc(
                    ld_sems[g], 16
                )
            scalar.dma_start(
                out=abuf_t[:, 15, 0:HF], in_=a_hl[30]
            ).then_inc(eg_sems[(15, 0)], 16)
            for q in (1, 2):
                o, n = EG_OFF[q]
                scalar.dma_start(
                    out=abuf_t[:, 15, o : o + n], in_=a_q[60 + 1 + q]
                ).then_inc(eg_sems[(15, q)], 16)
            # ACT also bounces finished psum rows to SBUF (DMA cannot
            # read PSUM). Batch 14 completes at item 19, batch 15 at 20.
            for g in range(NSLAB):
                off = 32 * (g % 2)
                n_items = (g + 1) if g < 14 else (19 + (g - 14))
                scalar.wait_ge(pe_sem, n_items)
                scalar.copy(
                    ost[:, g * D : (g + 1) * D], psb[g // 2][off : off + 1, :]
                ).then_inc(cp_sem, 1)

        @block.vector
        def _(vector):
            for i, item in enumerate(CAST_ORDER):
                slot = i % NSLOT
                if i >= NSLOT:
                    # Slot reuse: wait until item i-NSLOT is fully
                    # consumed by the PE.
                    vector.wait_ge(pe_sem, i - NSLOT + 1)
                if isinstance(item, int):
                    g = item
                    vector.wait_ge(ld_sems[g], 16)
                    src = abuf_t[:, g]
                    dst = bbuf_t[:, slot, 0:FPP]
                else:
                    g, q = item
                    vector.wait_ge(eg_sems[(g, q)], 16)
                    o, n = EG_OFF[q]
                    src = abuf_t[:, g, o : o + n]
                    dst = bbuf_t[:, slot, 0:n]
                vector.tensor_copy(dst, src).then_inc(vred, 1)

        @block.tensor
        def _(tensor):
            for i, item in enumerate(CAST_ORDER):
                slot = i % NSLOT
                tensor.wait_ge(vred, i + 1)
                if isinstance(item, int):
                    g = item
                    pieces = [(j * D, j == 0, j == 3) for j in range(4)]
                else:
                    g, q = item
                    if q == 0:
                        pieces = [(0, True, False), (D, False, False)]
                    else:
                        pieces = [(0, False, q == 2)]
                off = 32 * (g % 2)
                for o, first, last in pieces:
                    mm = tensor.matmul(
                        psb[g // 2][off : off + 1, :],
                        lhsT=ones[:, 0:1],
                        rhs=bbuf_t[:, slot, o : o + D],
                        start=first,
                        stop=last,
                    )
                mm.then_inc(pe_sem, 1)

    return nc


def _get_bass():
    if "nc" not in _CACHE:
        _CACHE["nc"] = _build_bass()
    return _CACHE["nc"]


def run_spmd(a, **spmd_kwargs):
    """Run the SPMD kernel on all 8 cores; returns (full_output, BassKernelResults)."""
    from concourse.bass_utils import run_bass_kernel_spmd

    nc = _get_bass()
    a = np.ascontiguousarray(np.asarray(a), dtype=np.float32)
    assert a.shape == (B, TX, D), a.shape
    in_maps = [{"a": a[k * NB : (k + 1) * NB]} for k in range(N_CORES)]
    res = run_bass_kernel_spmd(nc, in_maps, list(range(N_CORES)), **spmd_kwargs)
    out = np.concatenate([res.results[k]["out"] for k in range(N_CORES)], axis=0)
    return out.reshape(B, 1, D).astype(np.float32), res


def kernel(a, s_prev=None, W1=None, b1=None, W2=None, b2=None, **_unused):
    out, _ = run_spmd(a)
    return out


# revision 5
# speedup vs baseline: 1.1288x; 1.0018x over previous
"""Trainium2 Bass kernel for an attention layer whose math collapses.

The module computes softmax over a size-1 axis, so the attention weights
are exactly 1.0 and the output is context[b, 0, d] = sum_t a[b, t, d].
The MLP branch (W1, b1, W2, b2) and s_prev never affect the output.

Strategy: pure data parallel over the batch axis; each of the 8 cores
reduces its [16, 512, 512] shard over the time axis. Memory-bound:
~16 MiB HBM read per core; trace-measured steady-state DMA rate with
full-width transfers is ~420 GB/s (both HWDGE rings interleaving onto
all 16 SDMA engines / SBUF AXI ports) -> ~40 us byte stream plus fixed
framework pre/postamble and a short tail.

Kernel shape (per core), tuned against perfetto traces:
  - 14 slabs of 1 MiB (one batch each) + a 3-piece endgame slab per
    ring, DMA'd as [128 partitions x contiguous bytes]. Even batches on
    the SP HWDGE ring, odd batches on the Activation ring. Every
    transfer spans all 128 partitions: 32-partition "early start"
    pieces only reach 8 of the 16 SBUF AXI ports and run the first
    ~12 us of the stream at half rate (trace-verified), so they are
    never used.
  - Per slab, DVE does ONE fp32 -> bf16 cast (~1.2 us) into a rotating
    4-slot bf16 scratch (a previous in-place bitcast variant raced the
    cast's read stream and corrupted results; the scratch slot is
    guarded by the matmul-progress sem). No fold adds: the PE reduces
    both the partition axis and the 4-rows-per-partition axis with 4
    accumulating bf16 matmuls (ones [128,1] lhsT, N=512, fp32 PSUM
    accum). bf16 keeps PE at ~0.4-0.6 us/matmul (fp32 would LOW/HIGH-
    split to ~2 us) and the rounding error (~5e-4 rel) is far inside
    the 2e-2 gate. GPSIMD does nothing: concurrent GpSimd tensor ops
    stall DVE ops 2-4x (shared-port lock, trace-verified).
  - The last batch of each ring (14 on SP, 15 on ACT) is loaded as a
    half then two quarter DMAs, so the post-last-byte serial chain is
    one 512-element cast (~0.35 us) + one matmul + psum bounce + store.
  - ACT bounces each psum row to SBUF (DMA cannot read PSUM); per-slab
    2 KiB stores overlap all but the last store's latency.
  - The ~7 us end-of-kernel postamble (walrus resets all 256 HW
    semaphores round-robin across engines, inside the measured window)
    is framework-fixed; kernel-side sem slimming cannot shrink it.

Raw Bass (not Tile): with per-DMA completion semaphores every wait is a
standalone single-condition instruction and Tile's tail barriers are
avoided.
"""

from contextlib import ExitStack

import numpy as np

B, TX, D = 128, 512, 512
N_CORES = 8
NB = B // N_CORES   # 16 batches per core
P = 128             # SBUF partitions
NSLAB = 16          # 1 MiB slab groups per core (= one batch per slab)
FPP = NB * TX * D // (NSLAB * P)  # f32 per partition per slab = 2048
NSLOT = 8           # rotating bf16 scratch slots

_CACHE: dict = {}


def _build_bass():
    import concourse.bass as bass
    import concourse.mybir as mybir

    f32 = mybir.dt.float32
    bf16 = mybir.dt.bfloat16
    nc = bass.Bass("TRN2")
    a = nc.dram_tensor("a", [NB, TX, D], f32, kind="ExternalInput")
    out = nc.dram_tensor("out", [NB, D], f32, kind="ExternalOutput")

    ones = nc.const_aps.aps[(bf16, 1.0)]  # preamble-initialized [128, 1]
    a_sl = a.rearrange("b t d -> (b t d)").rearrange(
        "(g p f) -> g p f", g=NSLAB, p=P
    )

    with ExitStack() as ctx:
        abuf = ctx.enter_context(nc.sbuf_tensor([P, NSLAB * FPP], f32))
        bbuf = ctx.enter_context(nc.sbuf_tensor([P, NSLOT * FPP], bf16))
        ost = ctx.enter_context(nc.sbuf_tensor([1, NB * D], f32))
        psb = [
            ctx.enter_context(nc.psum_tensor(f"ps{i}", [64, D], f32))
            for i in range(8)
        ]
        # One completion semaphore per load DMA: concurrent DMA
        # completions are unordered, so a shared counting sem is racy.
        ld_sems = [
            ctx.enter_context(nc.semaphore(f"ld_sem{g}"))
            for g in range(NSLAB - 2)
        ]
        # Endgame pieces: (batch, piece) with piece 0 = half (1024 f32),
        # pieces 1, 2 = quarters (512 f32 each).
        eg_sems = {
            (g, q): ctx.enter_context(nc.semaphore(f"eg{g}_{q}"))
            for g in (14, 15)
            for q in (0, 1, 2)
        }
        # DVE cast counter (program-order), matmul item counter (one inc
        # per fully-matmul'd cast item, guards scratch-slot reuse and
        # gates bounces), bounce counter, store counter (all stores inc
        # one sem; the final wait needs the exact total 16*16, which is
        # order-independent).
        vred = ctx.enter_context(nc.semaphore("vred"))
        pe_sem = ctx.enter_context(nc.semaphore("pe_sem"))
        cp_sem = ctx.enter_context(nc.semaphore("cp_sem"))
        st_sem = ctx.enter_context(nc.semaphore("st_sem"))
        block = ctx.enter_context(nc.Block(no_gpsimd_drain=True))

        abuf_t = abuf[:].rearrange("p (g f) -> p g f", g=NSLAB)
        bbuf_t = bbuf[:].rearrange("p (s f) -> p s f", s=NSLOT)
        HF = FPP // 2   # 1024
        QF = FPP // 4   # 512
        # Endgame piece offsets/lengths in f32 elements within the slab.
        EG_OFF = {0: (0, HF), 1: (HF, QF), 2: (HF + QF, QF)}
        a_q = a.rearrange("b t d -> (b t d)").rearrange(
            "(x p f) -> x p f", x=4 * NSLAB, p=P
        )  # 256 KiB quarters: slab g = quarters 4g..4g+3
        a_hl = a.rearrange("b t d -> (b t d)").rearrange(
            "(x p f) -> x p f", x=2 * NSLAB, p=P
        )  # 512 KiB halves: slab g = halves 2g, 2g+1
        # DVE cast order = approximate arrival order; PE and the slot-
        # reuse guard follow the same order via counting sems.
        CAST_ORDER = list(range(14)) + [
            (14, 0), (15, 0), (14, 1), (15, 1), (14, 2), (15, 2)
        ]

        @block.sync
        def _(sync):
            for g in range(0, NSLAB - 2, 2):
                sync.dma_start(out=abuf_t[:, g], in_=a_sl[g]).then_inc(ld_sems[g], 16)
            sync.dma_start(
                out=abuf_t[:, 14, 0:HF], in_=a_hl[28]
            ).then_inc(eg_sems[(14, 0)], 16)
            for q in (1, 2):
                o, n = EG_OFF[q]
                sync.dma_start(
                    out=abuf_t[:, 14, o : o + n], in_=a_q[56 + 1 + q]
                ).then_inc(eg_sems[(14, q)], 16)
            # Per-slab 2 KiB stores: all but the last store's latency
            # overlaps with remaining compute.
            for g in range(NSLAB):
                sync.wait_ge(cp_sem, g + 1)
                sync.dma_start(
                    out=out[g : g + 1, :], in_=ost[0:1, g * D : (g + 1) * D]
                ).then_inc(st_sem, 16)
            sync.wait_ge(st_sem, 16 * NSLAB)

        @block.scalar
        def _(scalar):
            # Second HWDGE ring (Activation sequencer) for the odd slabs.
            for g in range(1, NSLAB - 2, 2):
                scalar.dma_start(out=abuf_t[:, g], in_=a_sl[g]).then_inc(
                    ld_sems[g], 16
                )
            scalar.dma_start(
                out=abuf_t[:, 15, 0:HF], in_=a_hl[30]
            ).then_inc(eg_sems[(15, 0)], 16)
            for q in (1, 2):
                o, n = EG_OFF[q]
                scalar.dma_start(
                    out=abuf_t[:, 15, o : o + n], in_=a_q[60 + 1 + q]
                ).then_inc(eg_sems[(15, q)], 16)
            # ACT also bounces finished psum rows to SBUF (DMA cannot
            # read PSUM). Batch 14 completes at item 19, batch 15 at 20.
            for g in range(NSLAB):
                off = 32 * (g % 2)
                n_items = (g + 1) if g < 14 else (19 + (g - 14))
                scalar.wait_ge(pe_sem, n_items)
                scalar.copy(
                    ost[:, g * D : (g + 1) * D], psb[g // 2][off : off + 1, :]
                ).then_inc(cp_sem, 1)

        @block.vector
        def _(vector):
            for i, item in enumerate(CAST_ORDER):
                slot = i % NSLOT
                if i >= NSLOT:
                    # Slot reuse: wait until item i-NSLOT is fully
                    # consumed by the PE.
                    vector.wait_ge(pe_sem, i - NSLOT + 1)
                if isinstance(item, int):
                    g = item
                    vector.wait_ge(ld_sems[g], 16)
                    src = abuf_t[:, g]
                    dst = bbuf_t[:, slot, 0:FPP]
                else:
                    g, q = item
                    vector.wait_ge(eg_sems[(g, q)], 16)
                    o, n = EG_OFF[q]
                    src = abuf_t[:, g, o : o + n]
                    dst = bbuf_t[:, slot, 0:n]
                vector.tensor_copy(dst, src).then_inc(vred, 1)

        @block.tensor
        def _(tensor):
            for i, item in enumerate(CAST_ORDER):
                slot = i % NSLOT
                tensor.wait_ge(vred, i + 1)
                if isinstance(item, int):
                    g = item
                    pieces = [(j * D, j == 0, j == 3) for j in range(4)]
                else:
                    g, q = item
                    if q == 0:
                        pieces = [(0, True, False), (D, False, False)]
                    else:
                        pieces = [(0, False, q == 2)]
                off = 32 * (g % 2)
                for o, first, last in pieces:
                    mm = tensor.matmul(
                        psb[g // 2][off : off + 1, :],
                        lhsT=ones[:, 0:1],
                        rhs=bbuf_t[:, slot, o : o + D],
                        start=first,
                        stop=last,
                    )
                mm.then_inc(pe_sem, 1)

    return nc


def _get_bass():
    if "nc" not in _CACHE:
        _CACHE["nc"] = _build_bass()
    return _CACHE["nc"]


def run_spmd(a, **spmd_kwargs):
    """Run the SPMD kernel on all 8 cores; returns (full_output, BassKernelResults)."""
    from concourse.bass_utils import run_bass_kernel_spmd

    nc = _get_bass()
    a = np.ascontiguousarray(np.asarray(a), dtype=np.float32)
    assert a.shape == (B, TX, D), a.shape
    in_maps = [{"a": a[k * NB : (k + 1) * NB]} for k in range(N_CORES)]
    res = run_bass_kernel_spmd(nc, in_maps, list(range(N_CORES)), **spmd_kwargs)
    out = np.concatenate([res.results[k]["out"] for k in range(N_CORES)], axis=0)
    return out.reshape(B, 1, D).astype(np.float32), res


def kernel(a, s_prev=None, W1=None, b1=None, W2=None, b2=None, **_unused):
    out, _ = run_spmd(a)
    return out
